# revision 1
# baseline (speedup 1.0000x reference)
"""EntNet Trainium2 kernel.

B=32, S=256, L=64, D=100, M=20. Data-parallel over batch: 8 cores x B_loc=4.

Per core:
  Phase 1 (encode, memory-bound): stream batch slice [4, 256*64, 100] in
    contiguous [128, 32, 100] tiles; (optionally) multiply by enc_mult
    pattern; DVE segmented-reduce over the 32 rows-per-partition; PE
    selector-matmul to finish the word reduction across partitions and land
    the result directly transposed as encT [100 (d), 256 (t), 4 (b)].
  Precompute (per 64-sentence chunk, overlapped with the scan):
    kg_all[bm, t] = sum_d keys[m,d] * enc[b,t,d]   (key gate, all steps)
    ws_all[(t b), e] = sum_d enc[b,t,d] * W[e,d]   (W s term, all steps)
  Phase 2 (scan, latency-bound): 256 sequential steps on state
    mem [80 (b*20+m), 100] and its transpose memT [100, 80]:
      g4   = memT.T @ sT_t                         (PE, [80,4])
      cand = memT.T @ UT + SELB.T @ ws_t + SELM.T @ keysV   (PE psum accum)
      gate_pre = sum_free(g4 * MASK) + kg_t        (DVE ttr, [80,1])
      gate = sigmoid(gate_pre)                     (ACT)
      mem' = (cand * gate) + mem                   (DVE stt, prelu_a==1 path)
      ssq  = sum(mem'^2)                           (ACT square+accum)
      inv  = 1/sqrt(ssq)                           (ACT sqrt + DVE recip)
      mem_new = mem' * inv                         (ACT copy w/ scale)
      memT_new = transpose(mem_new)                (PE) + copy to SBUF (DVE)
"""

import os
from contextlib import ExitStack

import numpy as np

B, S, L, D, M = 32, 256, 64, 100, 20
NCORES = 8
BL = B // NCORES          # 4 batches per core
BM = BL * M               # 80 state rows per core
RPP = 32                  # (s,l) rows per partition in encode tiles
TILE_ROWS = 128 * RPP     # 4096 rows per encode tile
NCHUNK = (S * L) // TILE_ROWS   # 4 encode tiles per b
S_PER_TILE = TILE_ROWS // L     # 64 sentences per encode tile
WS_CHUNKS = (S * BL) // 128     # 8 column chunks of ws_all

_built = {}


def _build(apply_mult: bool, a_is_one: bool, a: float, n_steps: int = S):
    import concourse.bacc as bacc
    import concourse.bass as bass
    import concourse.tile as tile
    import concourse.mybir as mybir

    f32 = mybir.dt.float32
    Alu = mybir.AluOpType
    Act = mybir.ActivationFunctionType

    nc = bacc.Bacc("TRN2", target_bir_lowering=False, debug=False)

    x = nc.dram_tensor("x", [BL, S * L, D], f32, kind="ExternalInput").ap()
    c_ut = nc.dram_tensor("c_ut", [D, D], f32, kind="ExternalInput").ap()
    c_wt = nc.dram_tensor("c_wt", [D, D], f32, kind="ExternalInput").ap()
    c_keyst = nc.dram_tensor("c_keyst", [D, M], f32, kind="ExternalInput").ap()
    c_keysv = nc.dram_tensor("c_keysv", [M, D], f32, kind="ExternalInput").ap()
    c_selb = nc.dram_tensor("c_selb", [BL, BM], f32, kind="ExternalInput").ap()
    c_selm = nc.dram_tensor("c_selm", [M, BM], f32, kind="ExternalInput").ap()
    c_mask = nc.dram_tensor("c_mask", [BM, BL], f32, kind="ExternalInput").ap()
    c_mem0 = nc.dram_tensor("c_mem0", [BM, D], f32, kind="ExternalInput").ap()
    c_memt0 = nc.dram_tensor("c_memt0", [D, BM], f32, kind="ExternalInput").ap()
    c_id80 = nc.dram_tensor("c_id80", [BM, BM], f32, kind="ExternalInput").ap()
    c_sel = nc.dram_tensor("c_sel", [128, S_PER_TILE], f32, kind="ExternalInput").ap()
    if apply_mult:
        c_pat = nc.dram_tensor("c_pat", [128, RPP, D], f32, kind="ExternalInput").ap()
    out = nc.dram_tensor("out", [BM, D], f32, kind="ExternalOutput").ap()

    with tile.TileContext(nc) as tc, ExitStack() as ctx:
        consts = ctx.enter_context(tc.tile_pool(name="consts", bufs=1))
        persist = ctx.enter_context(tc.tile_pool(name="persist", bufs=1))
        enc_in = ctx.enter_context(tc.tile_pool(name="enc_in", bufs=3))
        work = ctx.enter_context(tc.tile_pool(name="work", bufs=3))
        state = ctx.enter_context(tc.tile_pool(name="state", bufs=3))
        ps_enc = ctx.enter_context(tc.tile_pool(name="ps_enc", bufs=2, space="PSUM"))
        ps_cand = ctx.enter_context(tc.tile_pool(name="ps_cand", bufs=2, space="PSUM"))
        ps_g4 = ctx.enter_context(tc.tile_pool(name="ps_g4", bufs=2, space="PSUM"))
        ps_t = ctx.enter_context(tc.tile_pool(name="ps_t", bufs=2, space="PSUM"))

        def load_const(ap, shape, tag):
            t = consts.tile(shape, f32, tag=tag)
            nc.sync.dma_start(t, ap)
            return t

        ut_sb = load_const(c_ut, [D, D], "ut")
        wt_sb = load_const(c_wt, [D, D], "wt")
        keyst_sb = load_const(c_keyst, [D, M], "keyst")
        keysv_sb = load_const(c_keysv, [M, D], "keysv")
        selb_sb = load_const(c_selb, [BL, BM], "selb")
        selm_sb = load_const(c_selm, [M, BM], "selm")
        mask_sb = load_const(c_mask, [BM, BL], "mask")
        id80_sb = load_const(c_id80, [BM, BM], "id80")
        sel_sb = load_const(c_sel, [128, S_PER_TILE], "sel")
        if apply_mult:
            pat_sb = load_const(c_pat, [128, RPP, D], "pat")

        encT = persist.tile([D, BL, S], f32)       # [100, 4, 256]
        kg_sb = persist.tile([BM, S], f32)         # [80, 256]
        ws_b = persist.tile([BL, S, D], f32)       # [4, 256, 100]

        mem = state.tile([BM, D], f32, tag="mem")
        memT = state.tile([D, BM], f32, tag="memT")
        nc.sync.dma_start(mem, c_mem0)
        nc.sync.dma_start(memT, c_memt0)

        # ---- Phase 1: encode, chunked by 64-sentence groups so the scan
        # can start as soon as the first chunk lands.
        for c in range(NCHUNK):
            for b in range(BL):
                xt = enc_in.tile([128, RPP, D], f32, tag="xt")
                nc.sync.dma_start(
                    xt,
                    x[b, c * TILE_ROWS:(c + 1) * TILE_ROWS, :].rearrange(
                        "(p r) d -> p r d", p=128
                    ),
                )
                if apply_mult:
                    nc.vector.tensor_mul(xt, xt, pat_sb)
                red = enc_in.tile([128, D], f32, tag="red")
                nc.vector.tensor_reduce(
                    red,
                    xt[:].rearrange("p r d -> p d r"),
                    axis=mybir.AxisListType.X,
                    op=Alu.add,
                )
                ep = ps_enc.tile([D, S_PER_TILE], f32, tag="encps")
                nc.tensor.matmul(ep, lhsT=red, rhs=sel_sb, start=True, stop=True)
                nc.scalar.copy(encT[:, b, c * S_PER_TILE:(c + 1) * S_PER_TILE], ep)

            # key-gate chunk: kg[b*20+m, t] = sum_d keys[m,d] enc[b,t,d].
            # PSUM matmul outputs must start at partition 0/32/64, so compute
            # per-b [20, 64] tiles and DMA them to their partition offset.
            for b in range(BL):
                kp = ps_enc.tile([M, S_PER_TILE], f32, tag="encps")
                nc.tensor.matmul(
                    kp,
                    lhsT=keyst_sb,
                    rhs=encT[:, b, c * S_PER_TILE:(c + 1) * S_PER_TILE],
                    start=True,
                    stop=True,
                )
                kb = enc_in.tile([M, S_PER_TILE], f32, tag="kb")
                nc.scalar.copy(kb, kp)
                nc.sync.dma_start(
                    kg_sb[b * M:(b + 1) * M, c * S_PER_TILE:(c + 1) * S_PER_TILE],
                    kb,
                )

            # W s chunks: ws_b[b, t, e] = sum_d enc[b,t,d] W[e,d], 32 t at a time
            for cc in range(2 * c, 2 * c + 2):
                for b in range(BL):
                    wp = ps_enc.tile([32, D], f32, tag="encps")
                    nc.tensor.matmul(
                        wp,
                        lhsT=encT[:, b, cc * 32:(cc + 1) * 32],
                        rhs=wt_sb,
                        start=True,
                        stop=True,
                    )
                    wb = enc_in.tile([32, D], f32, tag="wb")
                    nc.scalar.copy(wb, wp)
                    nc.sync.dma_start(ws_b[b:b + 1, cc * 32:(cc + 1) * 32, :], wb)


        # ---- Phase 2: the scan.
        for t in range(n_steps):
            sT = encT[:, :, t]  # [100, 4] (stride S between b columns)

            g4 = ps_g4.tile([BM, BL], f32, tag="g4")
            nc.tensor.matmul(g4, lhsT=memT, rhs=sT, start=True, stop=True)

            cand = ps_cand.tile([BM, D], f32, tag="cand")
            nc.tensor.matmul(cand, lhsT=memT, rhs=ut_sb, start=True, stop=False)
            # W s term broadcast over m via selector matmul
            nc.tensor.matmul(
                cand, lhsT=selb_sb, rhs=ws_b[:, t, :], start=False, stop=False
            )
            nc.tensor.matmul(cand, lhsT=selm_sb, rhs=keysv_sb, start=False, stop=True)

            g_scr = work.tile([BM, BL], f32, tag="gscr")
            gpre = work.tile([BM, 1], f32, tag="gpre")
            nc.vector.tensor_mul(g_scr, g4, mask_sb)
            nc.vector.tensor_reduce(
                gpre, g_scr, axis=mybir.AxisListType.X, op=Alu.add
            )
            gate = work.tile([BM, 1], f32, tag="gate")
            nc.scalar.activation(
                gate, gpre, func=Act.Sigmoid, bias=kg_sb[:, t:t + 1]
            )

            mem_pre = work.tile([BM, D], f32, tag="mem_pre")
            if a_is_one:
                # prelu is identity: mem' = cand*gate + mem in one op
                nc.vector.scalar_tensor_tensor(
                    out=mem_pre, in0=cand, scalar=gate, in1=mem,
                    op0=Alu.mult, op1=Alu.add,
                )
            else:
                pos = work.tile([BM, D], f32, tag="pos")
                nc.vector.tensor_scalar(
                    out=pos, in0=cand, scalar1=0.0, scalar2=gate,
                    op0=Alu.max, op1=Alu.mult,
                )
                neg = work.tile([BM, D], f32, tag="neg")
                nc.vector.tensor_scalar(
                    out=neg, in0=cand, scalar1=0.0, scalar2=gate,
                    op0=Alu.min, op1=Alu.mult,
                )
                tmp = work.tile([BM, D], f32, tag="tmp")
                nc.vector.scalar_tensor_tensor(
                    out=tmp, in0=neg, scalar=float(a), in1=pos,
                    op0=Alu.mult, op1=Alu.add,
                )
                nc.vector.tensor_add(mem_pre, tmp, mem)

            sq_scr = work.tile([BM, D], f32, tag="sq_scr")
            ssq = work.tile([BM, 1], f32, tag="ssq")
            nc.scalar.activation(sq_scr, mem_pre, func=Act.Square, accum_out=ssq)
            nrm = work.tile([BM, 1], f32, tag="nrm")
            nc.scalar.activation(nrm, ssq, func=Act.Sqrt)
            inv = work.tile([BM, 1], f32, tag="inv")
            nc.vector.reciprocal(inv, nrm)

            mem_new = state.tile([BM, D], f32, tag="mem")
            nc.scalar.mul(mem_new, mem_pre, inv)

            mt_ps = ps_t.tile([D, BM], f32, tag="mtps")
            nc.tensor.transpose(mt_ps, mem_new, id80_sb)
            memT_new = state.tile([D, BM], f32, tag="memT")
            nc.vector.tensor_copy(memT_new, mt_ps)

            mem, memT = mem_new, memT_new

        nc.sync.dma_start(out, mem)

    nc.compile()
    return nc


def _consts(enc_mult, keys, U, V, W, apply_mult):
    f = np.float32
    keys = np.asarray(keys, f)
    U = np.asarray(U, f)
    V = np.asarray(V, f)
    W = np.asarray(W, f)
    enc_mult = np.asarray(enc_mult, f)

    selm = np.zeros((M, BM), f)
    for bm in range(BM):
        selm[bm % M, bm] = 1.0
    selb = np.zeros((BL, BM), f)
    for bm in range(BM):
        selb[bm // M, bm] = 1.0
    mask = np.zeros((BM, BL), f)
    for bm in range(BM):
        mask[bm, bm // M] = 1.0
    sel = np.zeros((128, S_PER_TILE), f)
    for p in range(128):
        sel[p, p // (L // RPP)] = 1.0

    c = {
        "c_ut": np.ascontiguousarray(U.T),
        "c_wt": np.ascontiguousarray(W.T),
        "c_keyst": np.ascontiguousarray(keys.T),
        "c_keysv": np.ascontiguousarray(keys @ V.T),
        "c_selb": selb,
        "c_selm": selm,
        "c_mask": mask,
        "c_mem0": np.ascontiguousarray(np.tile(keys, (BL, 1))),
        "c_memt0": np.ascontiguousarray(np.tile(keys.T, (1, BL))),
        "c_id80": np.eye(BM, dtype=f),
        "c_sel": sel,
    }
    if apply_mult:
        pat = np.empty((128, RPP, D), f)
        for p in range(128):
            for r in range(RPP):
                pat[p, r, :] = enc_mult[(p * RPP + r) % L, :]
        c["c_pat"] = pat
    return c


def kernel(batch, enc_mult, keys, U, V, W, prelu_a):
    from concourse.bass_utils import run_bass_kernel_spmd

    batch = np.ascontiguousarray(np.asarray(batch, np.float32))
    enc_mult = np.asarray(enc_mult, np.float32)
    a = float(np.asarray(prelu_a))
    apply_mult = not bool(np.all(enc_mult == 1.0))
    a_is_one = a == 1.0

    key = (apply_mult, a_is_one, a)
    if key not in _built:
        _built[key] = _build(apply_mult, a_is_one, a)
    nc = _built[key]

    consts = _consts(enc_mult, keys, U, V, W, apply_mult)
    in_maps = []
    for cidx in range(NCORES):
        m = dict(consts)
        m["x"] = np.ascontiguousarray(
            batch[cidx * BL:(cidx + 1) * BL].reshape(BL, S * L, D)
        )
        in_maps.append(m)

    trace = os.environ.get("ENTNET_TRACE", "") == "1"
    res = run_bass_kernel_spmd(
        nc, in_maps, core_ids=list(range(NCORES)), trace=trace
    )
    if trace:
        print(f"HW exec time: {res.exec_time_ns} ns")
        if res.instructions_and_trace is not None:
            print(f"trace: {res.instructions_and_trace[1]}")

    return np.concatenate(
        [r["out"].reshape(BL, M, D) for r in res.results], axis=0
    )

